# revision 10
# baseline (speedup 1.0000x reference)
"""Multi-head causal self-attention on 8 Trainium2 NeuronCores.

Sharding: tensor-parallel over heads (4 heads/core) x data-parallel over
batch (B=2): core c -> batch c//4, head-group c%4. Each core computes its
4 heads' attention plus a partial output projection; the host sums the 4
partials per batch element.

Layout strategy (per core):
  - x is fed pre-transposed (xT: [D, T]) so QKV projections produce
    qT/kT ([head_dim, T], head-dim on partitions) and v ([T, head_dim])
    directly, with no on-device transposes anywhere.
  - Scores are computed transposed (k on partitions, q on free dim):
    psum[k, q] = kT_tile.T @ qT_block. Two heads run concurrently via
    row-tiled tile_position (dk=64 each).
  - Softmax skips max-subtraction (scores are bounded well inside fp32
    exp range); exp runs on ScalarE with scale=1/sqrt(dk) folded in.
    Causal masking multiplies only diagonal tiles by a 0/1 mask.
  - P@V runs col-tiled (M=64) with a concurrent M=1 ones-matmul in a
    spare column group accumulating the softmax denominators.
  - Normalization is applied after P@V: denominators are reciprocated,
    broadcast across partitions by a K=1 matmul, and multiplied in.
  - All matmul operands are float32r (single-pass FP22, full PE rate).
"""

import sys

for _p in ("/opt/trn_rl_repo",):
    if _p not in sys.path:
        sys.path.append(_p)

import numpy as np

P = 128
T = 2048
D = 1024
OD = 256  # output dims per core = 4 heads x 64
DK = 64
NQ = 512  # q-block (psum free size)
N_CORES = 8

_CACHE = {}


def _build_nc(t=T, d=D, od=OD):
    import concourse.bass as bass
    import concourse.tile as tile
    from concourse import bacc, mybir

    f32 = mybir.dt.float32
    f32r = mybir.dt.float32r

    kt = d // P        # k-tiles over d_model
    tt = t // P        # token tiles
    nb = t // NQ       # q blocks
    npair = od // P    # head pairs (2 heads per 128 partitions)
    dpb = NQ // P      # diagonal k-tiles per q block

    nc = bacc.Bacc("TRN2", target_bir_lowering=False, debug=False)

    xT = nc.dram_tensor("xT", [d, t], f32r, kind="ExternalInput")
    wqT = nc.dram_tensor("wqT", [d, od], f32r, kind="ExternalInput")
    wkT = nc.dram_tensor("wkT", [d, od], f32r, kind="ExternalInput")
    wvT = nc.dram_tensor("wvT", [d, od], f32r, kind="ExternalInput")
    woT = nc.dram_tensor("woT", [od, d], f32r, kind="ExternalInput")
    masks = nc.dram_tensor("masks", [P, dpb * NQ], mybir.dt.bfloat16, kind="ExternalInput")
    y = nc.dram_tensor("y", [t, d], f32, kind="ExternalOutput")

    Exp = mybir.ActivationFunctionType.Exp
    scale = 1.0 / float(np.sqrt(DK))

    with tile.TileContext(nc) as tc:
        with (
            tc.tile_pool(name="const", bufs=1) as cpool,
            tc.tile_pool(name="qk", bufs=2 * npair * nb) as qkpool,
            tc.tile_pool(name="vp", bufs=tt) as vpool,
            tc.tile_pool(name="ht", bufs=npair * nb) as hpool,
            tc.tile_pool(name="psA", bufs=4, space="PSUM") as psA,
            tc.tile_pool(name="psH", bufs=2, space="PSUM") as psH,
            tc.tile_pool(name="psB", bufs=2, space="PSUM") as psB,
        ):
            # ---- constants / inputs ----
            wo_sb = cpool.tile([P, npair * d], f32r, tag="wo")
            for pp in range(npair):
                nc.sync.dma_start(wo_sb[:, pp * d:(pp + 1) * d], woT[pp * P:(pp + 1) * P, :])
            mask_sb = cpool.tile([P, dpb * NQ], mybir.dt.bfloat16, tag="mask")
            nc.sync.dma_start(mask_sb[:], masks[:])
            ones_sb = cpool.tile([P, DK], f32r, tag="ones")
            nc.vector.memset(ones_sb[:].bitcast(f32), 1.0)

            # x and the QKV weights live in a scoped pool released after the
            # projections, freeing 88KB/partition for the attention phase.
            xpool = tc.alloc_tile_pool(name="xp", bufs=1)
            x_sb = xpool.tile([P, kt * t], f32r, tag="x")
            for k in range(kt):
                nc.sync.dma_start(x_sb[:, k * t:(k + 1) * t], xT[k * P:(k + 1) * P, :])
            wq_sb = xpool.tile([P, kt * od], f32r, tag="wq")
            wk_sb = xpool.tile([P, kt * od], f32r, tag="wk")
            wv_sb = xpool.tile([P, kt * od], f32r, tag="wv")
            for k in range(kt):
                nc.sync.dma_start(wq_sb[:, k * od:(k + 1) * od], wqT[k * P:(k + 1) * P, :])
                nc.sync.dma_start(wk_sb[:, k * od:(k + 1) * od], wkT[k * P:(k + 1) * P, :])
                nc.sync.dma_start(wv_sb[:, k * od:(k + 1) * od], wvT[k * P:(k + 1) * P, :])

            # ---- QKV projections ----
            # qT/kT: [head-dim on partitions, tokens free], per (pair, q-block)
            qT = [[qkpool.tile([P, NQ], f32r, tag="qT", name=f"qT_{pp}_{n}") for n in range(nb)] for pp in range(npair)]
            kT = [[qkpool.tile([P, NQ], f32r, tag="kT", name=f"kT_{pp}_{n}") for n in range(nb)] for pp in range(npair)]
            for pp in range(npair):
                for n in range(nb):
                    for dst, w_sb in ((kT, wk_sb), (qT, wq_sb)):
                        ps = psA.tile([P, NQ], f32, tag="acc")
                        for k in range(kt):
                            nc.tensor.matmul(
                                ps[:],
                                w_sb[:, k * od + pp * P: k * od + (pp + 1) * P],
                                x_sb[:, k * t + n * NQ: k * t + (n + 1) * NQ],
                                start=(k == 0),
                                stop=(k == kt - 1),
                            )
                        nc.vector.tensor_copy(dst[pp][n][:], ps[:])
            # v: [tokens on partitions, head-dim free], per token-tile.
            # Each head's 64 columns are followed by a ones column so the
            # P@V matmul also accumulates the softmax denominator (M=65).
            nh = od // DK
            v_sb = [vpool.tile([P, od + nh], f32r, tag="v", name=f"v_{tk}") for tk in range(tt)]
            for tk in range(tt):
                nc.vector.memset(v_sb[tk][:].bitcast(f32), 1.0)
                ps = psA.tile([P, od], f32, tag="acc")
                for k in range(kt):
                    nc.tensor.matmul(
                        ps[:],
                        x_sb[:, k * t + tk * P: k * t + (tk + 1) * P],
                        wv_sb[:, k * od:(k + 1) * od],
                        start=(k == 0),
                        stop=(k == kt - 1),
                    )
                for hh in range(nh):
                    nc.vector.tensor_copy(
                        v_sb[tk][:, hh * (DK + 1): hh * (DK + 1) + DK],
                        ps[:, hh * DK:(hh + 1) * DK],
                    )

            xpool.release()
            wpool = tc.alloc_tile_pool(name="work", bufs=8)

            # ---- attention (per head pair, per q block) ----
            hT = [[hpool.tile([P, NQ], f32r, tag="hT", name=f"hT_{pp}_{n}") for n in range(nb)] for pp in range(npair)]
            for pp in range(npair):
                for j in range(nb):
                    nm = dpb * (j + 1)  # causal: k-tiles 0..nm-1
                    psh = [psH.tile([P, NQ], f32, tag="h", name=f"psh_{pp}_{j}_{h}") for h in range(2)]
                    for m in range(nm):
                        # scores (transposed): k on partitions, q free;
                        # both heads of the pair via row-tiled packing
                        es = []
                        for h in range(2):
                            pss = psA.tile([P, NQ], f32, tag="acc")
                            nc.tensor.matmul(
                                pss[:],
                                kT[pp][m // dpb][h * DK:(h + 1) * DK, (m % dpb) * P:(m % dpb + 1) * P],
                                qT[pp][j][h * DK:(h + 1) * DK, :],
                                start=True,
                                stop=True,
                                tile_position=(h * DK, 0),
                            )
                            e = wpool.tile([P, NQ], f32r, tag="exp")
                            nc.scalar.activation(e[:], pss[:], Exp, bias=0.0, scale=scale)
                            if m >= dpb * j:
                                dlt = m - dpb * j
                                nc.vector.tensor_mul(e[:], e[:], mask_sb[:, dlt * NQ:(dlt + 1) * NQ])
                            es.append(e)
                        for h in range(2):
                            hh = 2 * pp + h
                            nc.tensor.matmul(
                                psh[h][0:DK + 1, :],
                                v_sb[m][:, hh * (DK + 1): (hh + 1) * (DK + 1)],
                                es[h][:],
                                start=(m == 0),
                                stop=(m == nm - 1),
                            )
                    for h in range(2):
                        inv = wpool.tile([1, NQ], f32r, tag="inv", bufs=2)
                        with nc.allow_low_precision(reason="f32r shares f32 bits"):
                            nc.vector.reciprocal(inv[0:1, :], psh[h][DK:DK + 1, :])
                        psb = psB.tile([DK, NQ], f32, tag="b")
                        nc.tensor.matmul(psb[:], ones_sb[0:1, 0:DK], inv[0:1, :], start=True, stop=True)
                        bc = wpool.tile([DK, NQ], f32r, tag="bc", bufs=2)
                        nc.vector.tensor_copy(bc[:], psb[:])
                        nc.vector.tensor_mul(hT[pp][j][h * DK:(h + 1) * DK, :], psh[h][0:DK, :], bc[:])

            # ---- output projection (partial; host sums across head groups) ----
            obw = min(NQ, d)
            for tk in range(tt):
                for ob in range(d // obw):
                    psy = psA.tile([P, obw], f32, tag="acc")
                    for pp in range(npair):
                        nc.tensor.matmul(
                            psy[:],
                            hT[pp][tk // dpb][:, (tk % dpb) * P:(tk % dpb + 1) * P],
                            wo_sb[:, pp * d + ob * obw: pp * d + (ob + 1) * obw],
                            start=(pp == 0),
                            stop=(pp == npair - 1),
                        )
                    ysb = wpool.tile([P, obw], f32, tag="ysb", bufs=4)
                    nc.vector.tensor_copy(ysb[:], psy[:])
                    nc.sync.dma_start(y[tk * P:(tk + 1) * P, ob * obw:(ob + 1) * obw], ysb[:])
            wpool.release()

    nc.compile()
    return nc


def _get_nc():
    if "nc" not in _CACHE:
        _CACHE["nc"] = _build_nc()
    return _CACHE["nc"]


def _masks_np():
    import ml_dtypes
    kk = np.arange(P)[:, None]
    qq = np.arange(NQ)[None, :]
    return np.concatenate(
        [(kk <= qq - P * dlt) for dlt in range(NQ // P)], axis=1
    ).astype(ml_dtypes.bfloat16)


def make_in_maps(x, Wq, Wk, Wv, Wo):
    x = np.asarray(x, np.float32)
    msk = _masks_np()
    in_maps = []
    for c in range(N_CORES):
        b, g = c // (N_CORES // 2), c % (N_CORES // 2)
        hs = slice(OD * g, OD * (g + 1))
        in_maps.append({
            "xT": np.ascontiguousarray(x[b].T),
            "wqT": np.ascontiguousarray(np.asarray(Wq, np.float32)[hs, :].T),
            "wkT": np.ascontiguousarray(np.asarray(Wk, np.float32)[hs, :].T),
            "wvT": np.ascontiguousarray(np.asarray(Wv, np.float32)[hs, :].T),
            "woT": np.ascontiguousarray(np.asarray(Wo, np.float32)[:, hs].T),
            "masks": msk,
        })
    return in_maps


def combine_outputs(results):
    ng = N_CORES // 2
    out = np.empty((2, T, D), np.float32)
    for b in range(2):
        acc = results[b * ng]["y"].astype(np.float32)
        for g in range(1, ng):
            acc = acc + results[b * ng + g]["y"]
        out[b] = acc
    return out


def kernel(x, Wq, Wk, Wv, Wo):
    from concourse.bass_utils import run_bass_kernel_spmd

    nc = _get_nc()
    in_maps = make_in_maps(x, Wq, Wk, Wv, Wo)
    res = run_bass_kernel_spmd(nc, in_maps, list(range(N_CORES)))
    return combine_outputs(res.results)


# revision 14
# speedup vs baseline: 1.1253x; 1.1253x over previous
"""Multi-head causal self-attention on 8 Trainium2 NeuronCores.

Sharding: tensor-parallel over heads (4 heads/core) x data-parallel over
batch (B=2): core c -> batch c//4, head-group c%4. Each core computes its
4 heads' attention plus a partial output projection; the host sums the 4
partials per batch element.

Layout strategy (per core):
  - x is fed pre-transposed (xT: [D, T]) so QKV projections produce
    qT/kT ([head_dim, T], head-dim on partitions) and v ([T, head_dim])
    directly, with no on-device transposes anywhere.
  - Scores are computed transposed (k on partitions, q on free dim):
    psum[k, q] = kT_tile.T @ qT_block. Two heads run concurrently via
    row-tiled tile_position (dk=64 each).
  - Softmax skips max-subtraction (scores are bounded well inside fp32
    exp range); exp runs on ScalarE with scale=1/sqrt(dk) folded in.
    Causal masking multiplies only diagonal tiles by a 0/1 mask, and
    exp/mask/PV are restricted to the live column range there.
  - P@V uses an M=65 stationary [v_head | ones] so the softmax
    denominators accumulate in psum row 64 of the same matmul.
  - Normalization happens after P@V: denominators are collected across
    a pair's 8 (j, head) slots into one SBUF tile, reciprocated in a
    single partition-parallel op, broadcast across partitions by a K=1
    matmul, and multiplied in.
  - Matmul operands are bf16 (1 cycle/column on the PE; fp32r costs 2).
    Accumulation is fp32 in PSUM; denominators/reciprocals stay fp32.
"""

import sys

for _p in ("/opt/trn_rl_repo",):
    if _p not in sys.path:
        sys.path.append(_p)

import numpy as np

P = 128
T = 2048
D = 1024
OD = 256  # output dims per core = 4 heads x 64
DK = 64
NQ = 512  # q-block (psum free size)
N_CORES = 8

_CACHE = {}


def _build_nc(t=T, d=D, od=OD):
    import concourse.bass as bass
    import concourse.tile as tile
    from concourse import bacc, mybir

    f32 = mybir.dt.float32
    f32r = mybir.dt.float32r
    bf16 = mybir.dt.bfloat16

    kt = d // P        # k-tiles over d_model
    tt = t // P        # token tiles
    nb = t // NQ       # q blocks
    npair = od // P    # head pairs (2 heads per 128 partitions)
    dpb = NQ // P      # diagonal k-tiles per q block

    nc = bacc.Bacc("TRN2", target_bir_lowering=False, debug=False)

    xT = nc.dram_tensor("xT", [d, t], bf16, kind="ExternalInput")
    wqT = nc.dram_tensor("wqT", [d, od], bf16, kind="ExternalInput")
    wkT = nc.dram_tensor("wkT", [d, od], bf16, kind="ExternalInput")
    wvT = nc.dram_tensor("wvT", [d, od], bf16, kind="ExternalInput")
    woT = nc.dram_tensor("woT", [od, d], bf16, kind="ExternalInput")
    masks = nc.dram_tensor("masks", [P, dpb * NQ], bf16, kind="ExternalInput")
    emat = nc.dram_tensor("emat", [2 * (t // NQ), 2 * (t // NQ) * DK], f32r, kind="ExternalInput")
    y = nc.dram_tensor("y", [t, d], f32, kind="ExternalOutput")

    Exp = mybir.ActivationFunctionType.Exp
    scale = 1.0 / float(np.sqrt(DK))

    with tile.TileContext(nc) as tc:
        with (
            tc.tile_pool(name="const", bufs=1) as cpool,
            tc.tile_pool(name="qk", bufs=2 * npair * nb) as qkpool,
            tc.tile_pool(name="vp", bufs=tt) as vpool,
            tc.tile_pool(name="ht", bufs=npair * nb) as hpool,
            tc.tile_pool(name="hu", bufs=2 * nb) as hupool,
            tc.tile_pool(name="work", bufs=8) as wpool,
            tc.tile_pool(name="psA", bufs=4, space="PSUM") as psA,
            tc.tile_pool(name="psH", bufs=2, space="PSUM") as psH,
            tc.tile_pool(name="psB", bufs=2, space="PSUM") as psB,
        ):
            # ---- constants / inputs ----
            wo_sb = cpool.tile([P, npair * d], bf16, tag="wo")
            for pp in range(npair):
                nc.sync.dma_start(wo_sb[:, pp * d:(pp + 1) * d], woT[pp * P:(pp + 1) * P, :])
            mask_sb = cpool.tile([P, dpb * NQ], bf16, tag="mask")
            nc.sync.dma_start(mask_sb[:], masks[:])
            emat_sb = cpool.tile([2 * nb, 2 * nb * DK], f32r, tag="emat")
            nc.sync.dma_start(emat_sb[:], emat[:])

            # x and the QKV weights live in a scoped pool released after the
            # projections, freeing space for the attention phase.
            xpool = tc.alloc_tile_pool(name="xp", bufs=1)
            x_sb = [xpool.tile([P, t], bf16, tag=f"x{k}", name=f"x_{k}") for k in range(kt)]
            for k in range(kt):
                nc.sync.dma_start(x_sb[k][:], xT[k * P:(k + 1) * P, :])
            wq_sb = xpool.tile([P, kt * od], bf16, tag="wq")
            wk_sb = xpool.tile([P, kt * od], bf16, tag="wk")
            wv_sb = xpool.tile([P, kt * od], bf16, tag="wv")
            for k in range(kt):
                nc.sync.dma_start(wq_sb[:, k * od:(k + 1) * od], wqT[k * P:(k + 1) * P, :])
                nc.sync.dma_start(wk_sb[:, k * od:(k + 1) * od], wkT[k * P:(k + 1) * P, :])
                nc.sync.dma_start(wv_sb[:, k * od:(k + 1) * od], wvT[k * P:(k + 1) * P, :])

            # ---- QKV projections ----
            # qT/kT: [head-dim on partitions, tokens free], per (pair, q-block)
            qT = [[qkpool.tile([P, NQ], bf16, tag="qT", name=f"qT_{pp}_{n}") for n in range(nb)] for pp in range(npair)]
            kT = [[qkpool.tile([P, NQ], bf16, tag="kT", name=f"kT_{pp}_{n}") for n in range(nb)] for pp in range(npair)]
            for pp in range(npair):
                for n in range(nb):
                    for dst, w_sb in ((kT, wk_sb), (qT, wq_sb)):
                        ps = psA.tile([P, NQ], f32, tag="acc")
                        for k in range(kt):
                            nc.tensor.matmul(
                                ps[:],
                                w_sb[:, k * od + pp * P: k * od + (pp + 1) * P],
                                x_sb[k][:, n * NQ:(n + 1) * NQ],
                                start=(k == 0),
                                stop=(k == kt - 1),
                            )
                        nc.vector.tensor_copy(dst[pp][n][:], ps[:])
            # v: [tokens on partitions, head-dim free], per token-tile.
            # Each head's 64 columns are followed by a ones column so the
            # P@V matmul also accumulates the softmax denominator (M=65).
            nh = od // DK
            v_sb = [vpool.tile([P, nh * (DK + 1)], bf16, tag="v", name=f"v_{tk}") for tk in range(tt)]
            for tk in range(tt):
                nc.vector.memset(v_sb[tk][:], 1.0)
                ps = psA.tile([P, od], f32, tag="acc")
                for k in range(kt):
                    nc.tensor.matmul(
                        ps[:],
                        x_sb[k][:, tk * P:(tk + 1) * P],
                        wv_sb[:, k * od:(k + 1) * od],
                        start=(k == 0),
                        stop=(k == kt - 1),
                    )
                nc.vector.tensor_copy(
                    v_sb[tk][:].rearrange("p (h c) -> p h c", c=DK + 1)[:, :, 0:DK],
                    ps[:].rearrange("p (h c) -> p h c", c=DK),
                )
            xpool.release()

            # ---- attention (per head pair, per q block) ----
            hT = [[hpool.tile([P, NQ], bf16, tag="hT", name=f"hT_{pp}_{n}") for n in range(nb)] for pp in range(npair)]
            for pp in range(npair):
                # unnormalized P@V outputs and their denominators for this
                # pair's (j, head) slots; normalized together at pair end
                hu = [hupool.tile([DK, NQ], bf16, tag="hu", name=f"hu_{pp}_{n}") for n in range(2 * nb)]
                sums = wpool.tile([2 * nb, NQ], f32, tag="sums", bufs=2, name=f"sums_{pp}")
                for j in range(nb):
                    nm = dpb * (j + 1)  # causal: k-tiles 0..nm-1
                    psh = [psH.tile([P, NQ], f32, tag="h", name=f"psh_{pp}_{j}_{h}") for h in range(2)]
                    for m in range(nm):
                        dlt = m - dpb * j
                        lo = max(dlt, 0) * P  # first live q column of this k-tile
                        # scores (transposed): k on partitions, q free;
                        # both heads of the pair via row-tiled packing
                        es = []
                        for h in range(2):
                            pss = psA.tile([P, NQ], f32, tag="acc")
                            nc.tensor.matmul(
                                pss[:, lo:],
                                kT[pp][m // dpb][h * DK:(h + 1) * DK, (m % dpb) * P:(m % dpb + 1) * P],
                                qT[pp][j][h * DK:(h + 1) * DK, lo:],
                                start=True,
                                stop=True,
                                tile_position=(h * DK, 0),
                            )
                            e = wpool.tile([P, NQ], bf16, tag="exp")
                            nc.scalar.activation(e[:, lo:], pss[:, lo:], Exp, bias=0.0, scale=scale)
                            if dlt >= 0:
                                nc.vector.tensor_mul(e[:, lo:], e[:, lo:], mask_sb[:, dlt * NQ + lo:(dlt + 1) * NQ])
                            es.append(e)
                        for h in range(2):
                            hh = 2 * pp + h
                            nc.tensor.matmul(
                                psh[h][0:DK + 1, lo:],
                                v_sb[m][:, hh * (DK + 1): (hh + 1) * (DK + 1)],
                                es[h][:, lo:],
                                start=(m == 0),
                                stop=(m == nm - 1),
                            )
                    for h in range(2):
                        nc.vector.tensor_copy(hu[2 * j + h][:], psh[h][0:DK, :])
                        # DVE cannot write arbitrary start partitions; bounce
                        # the denominator row into the collector via DMA
                        stmp = wpool.tile([1, NQ], f32, tag="stmp", bufs=3)
                        nc.vector.tensor_copy(stmp[:], psh[h][DK:DK + 1, :])
                        nc.sync.dma_start(sums[2 * j + h:2 * j + h + 1, :], stmp[:])
                # one partition-parallel reciprocal for the whole pair
                inv = wpool.tile([2 * nb, NQ], f32r, tag="inv", bufs=2, name=f"inv_{pp}")
                with nc.allow_low_precision(reason="f32r shares f32 bits"):
                    nc.vector.reciprocal(inv[:], sums[:])
                for j in range(nb):
                    for h in range(2):
                        psb = psB.tile([DK, NQ], f32, tag="b")
                        i = 2 * j + h
                        nc.tensor.matmul(psb[:], emat_sb[:, i * DK:(i + 1) * DK], inv[:], start=True, stop=True)
                        nc.vector.tensor_mul(hT[pp][j][h * DK:(h + 1) * DK, :], hu[2 * j + h][:], psb[:])

            # ---- output projection (partial; host sums across head groups) ----
            obw = min(NQ, d)
            for tk in range(tt):
                for ob in range(d // obw):
                    psy = psA.tile([P, obw], f32, tag="acc")
                    for pp in range(npair):
                        nc.tensor.matmul(
                            psy[:],
                            hT[pp][tk // dpb][:, (tk % dpb) * P:(tk % dpb + 1) * P],
                            wo_sb[:, pp * d + ob * obw: pp * d + (ob + 1) * obw],
                            start=(pp == 0),
                            stop=(pp == npair - 1),
                        )
                    ysb = wpool.tile([P, obw], f32, tag="ysb", bufs=4)
                    nc.vector.tensor_copy(ysb[:], psy[:])
                    nc.sync.dma_start(y[tk * P:(tk + 1) * P, ob * obw:(ob + 1) * obw], ysb[:])

    nc.compile()
    return nc


def _get_nc():
    if "nc" not in _CACHE:
        _CACHE["nc"] = _build_nc()
    return _CACHE["nc"]


def _emat_np(nslots=2 * (T // NQ)):
    e = np.zeros((nslots, nslots * DK), np.float32)
    for i in range(nslots):
        e[i, i * DK:(i + 1) * DK] = 1.0
    return e


def _masks_np():
    import ml_dtypes
    kk = np.arange(P)[:, None]
    qq = np.arange(NQ)[None, :]
    return np.concatenate(
        [(kk <= qq - P * dlt) for dlt in range(NQ // P)], axis=1
    ).astype(ml_dtypes.bfloat16)


def make_in_maps(x, Wq, Wk, Wv, Wo):
    import ml_dtypes

    bf = ml_dtypes.bfloat16
    x = np.asarray(x, np.float32)
    msk = _masks_np()
    in_maps = []
    for c in range(N_CORES):
        b, g = c // (N_CORES // 2), c % (N_CORES // 2)
        hs = slice(OD * g, OD * (g + 1))
        in_maps.append({
            "xT": np.ascontiguousarray(x[b].T).astype(bf),
            "wqT": np.ascontiguousarray(np.asarray(Wq, np.float32)[hs, :].T).astype(bf),
            "wkT": np.ascontiguousarray(np.asarray(Wk, np.float32)[hs, :].T).astype(bf),
            "wvT": np.ascontiguousarray(np.asarray(Wv, np.float32)[hs, :].T).astype(bf),
            "woT": np.ascontiguousarray(np.asarray(Wo, np.float32)[:, hs].T).astype(bf),
            "masks": msk,
            "emat": _emat_np(),
        })
    return in_maps


def combine_outputs(results):
    ng = N_CORES // 2
    out = np.empty((2, T, D), np.float32)
    for b in range(2):
        acc = results[b * ng]["y"].astype(np.float32)
        for g in range(1, ng):
            acc = acc + results[b * ng + g]["y"]
        out[b] = acc
    return out


def kernel(x, Wq, Wk, Wv, Wo):
    from concourse.bass_utils import run_bass_kernel_spmd

    nc = _get_nc()
    in_maps = make_in_maps(x, Wq, Wk, Wv, Wo)
    res = run_bass_kernel_spmd(nc, in_maps, list(range(N_CORES)))
    return combine_outputs(res.results)
